# revision 2
# baseline (speedup 1.0000x reference)
"""2-layer LSTM (B=64, S=512, F=256, H=1024) on 8 Trainium2 NeuronCores.

Strategy: tensor-parallel over the 4H gate dimension. Core j owns hidden
units [j*128, (j+1)*128) of BOTH layers. On-chip layout: partition =
hidden unit, free = batch. All matmul operands are bf16 (PSUM/cell state
fp32) - 4x faster PE than fp32 and enables Fast Weight Load.

Per round r (software pipeline, L1 one step behind):
  - L0 computes step r:   gates0 = W_ih0 @ x_r + W_hh0 @ h0_{r-1}
    -> h0_r chunk -> AllGather AG0_r launched immediately
  - L1 computes step r-1: gates1 = W_hh1 @ h1_{r-2} + W_ih1 @ h0_{r-1}
    -> h1_{r-1} chunk -> AllGather AG1_{r-1}
Splitting the AllGather in two lets each one overlap the other layer's
compute instead of sitting exposed at the end of the round.
"""

import numpy as np

B, S, F, H = 64, 512, 256, 1024
P = 128
R = 8
NG = 4
KH = H // P   # 8
KF = F // P   # 2
# our gate order [i, f, o, g] -> torch row-block order [i, f, g, o]
TORCH_GATE = [0, 1, 3, 2]
# emission order for gates: i, f, g(tanh), o  (o last: only needed for final mul)
GATE_ORDER = [0, 1, 3, 2]

_CACHE = {}


def _build(nsteps, total=S):
    import concourse.bacc as bacc
    import concourse.mybir as mybir
    from concourse.tile import TileContext

    f32 = mybir.dt.float32
    bf = mybir.dt.bfloat16
    Sigmoid = mybir.ActivationFunctionType.Sigmoid
    Tanh = mybir.ActivationFunctionType.Tanh

    nc = bacc.Bacc("TRN2", num_devices=R, target_bir_lowering=False)
    xT = nc.dram_tensor("xT", [total, P, KF, B], bf, kind="ExternalInput")
    wih0 = nc.dram_tensor("wih0", [P, KF, NG, P], bf, kind="ExternalInput")
    whh0 = nc.dram_tensor("whh0", [P, KH, NG, P], bf, kind="ExternalInput")
    wih1 = nc.dram_tensor("wih1", [P, KH, NG, P], bf, kind="ExternalInput")
    whh1 = nc.dram_tensor("whh1", [P, KH, NG, P], bf, kind="ExternalInput")
    b0 = nc.dram_tensor("b0", [P, NG], f32, kind="ExternalInput")
    b1 = nc.dram_tensor("b1", [P, NG], f32, kind="ExternalInput")
    yT = nc.dram_tensor("yT", [total, P, B], bf, kind="ExternalOutput")
    cc0in = nc.dram_tensor("cc0in", [total, P, B], bf, kind="Internal")
    cc0out = nc.dram_tensor(
        "cc0out", [total, R * P, B], bf, kind="Internal", addr_space="Shared"
    )
    cc1in = nc.dram_tensor("cc1in", [total, P, B], bf, kind="Internal")
    cc1out = nc.dram_tensor(
        "cc1out", [total, R * P, B], bf, kind="Internal", addr_space="Shared"
    )
    rg = [list(range(R))]

    with TileContext(nc) as tc:
        with (
            tc.tile_pool(name="wpool", bufs=1) as wpool,
            tc.tile_pool(name="xin", bufs=6) as xin,
            tc.tile_pool(name="hbuf", bufs=4) as hbuf,
            tc.tile_pool(name="ew", bufs=3) as ew,
            tc.tile_pool(name="psum", bufs=4, space="PSUM") as pp,
        ):
            w_ih0 = wpool.tile([P, KF, NG, P], bf, tag="w0")
            nc.sync.dma_start(w_ih0[:], wih0[:])
            w_hh0 = wpool.tile([P, KH, NG, P], bf, tag="w1")
            nc.sync.dma_start(w_hh0[:], whh0[:])
            w_ih1 = wpool.tile([P, KH, NG, P], bf, tag="w2")
            nc.sync.dma_start(w_ih1[:], wih1[:])
            w_hh1 = wpool.tile([P, KH, NG, P], bf, tag="w3")
            nc.sync.dma_start(w_hh1[:], whh1[:])
            bias0 = wpool.tile([P, NG], f32, tag="b0")
            nc.sync.dma_start(bias0[:], b0[:])
            bias1 = wpool.tile([P, NG], f32, tag="b1")
            nc.sync.dma_start(bias1[:], b1[:])

            c0 = ew.tile([P, B], f32, tag="c0init")
            nc.vector.memset(c0[:], 0.0)
            c1 = ew.tile([P, B], f32, tag="c1init")
            nc.vector.memset(c1[:], 0.0)
            h0T = hbuf.tile([P, KH, B], bf, tag="h0T")
            nc.vector.memset(h0T[:], 0.0)
            h1T = hbuf.tile([P, KH, B], bf, tag="h1T")
            nc.vector.memset(h1T[:], 0.0)

            def act_chain(tagp, psum, bias, c, h_out):
                """gates (psum, fp32) + bias -> new c (fp32), h_out (bf16)."""
                i_sb = ew.tile([P, B], f32, tag=tagp + "i")
                f_sb = ew.tile([P, B], f32, tag=tagp + "f")
                o_sb = ew.tile([P, B], f32, tag=tagp + "o")
                g_sb = ew.tile([P, B], f32, tag=tagp + "g")
                sb = {0: i_sb, 1: f_sb, 2: o_sb, 3: g_sb}
                fn = {0: Sigmoid, 1: Sigmoid, 2: Sigmoid, 3: Tanh}
                ig = ew.tile([P, B], f32, tag=tagp + "ig")
                fc = ew.tile([P, B], f32, tag=tagp + "fc")
                c_new = ew.tile([P, B], f32, tag=tagp + "c")
                tc_sb = ew.tile([P, B], f32, tag=tagp + "tc")
                for g in GATE_ORDER:
                    nc.scalar.activation(
                        sb[g][:], psum[:, g, :], fn[g], bias=bias[:, g : g + 1]
                    )
                    if g == 1:
                        nc.vector.tensor_mul(fc[:], f_sb[:], c[:])
                    if g == 3:
                        nc.vector.tensor_mul(ig[:], i_sb[:], g_sb[:])
                        nc.vector.tensor_add(c_new[:], fc[:], ig[:])
                nc.scalar.activation(tc_sb[:], c_new[:], Tanh)
                nc.vector.tensor_mul(h_out[:], o_sb[:], tc_sb[:])
                return c_new

            for r in range(nsteps + 1):
                h0T_new = None
                h1T_new = None
                if r < nsteps:
                    # ---- L0 step r: uses h0T = gathered h0_{r-1} ----
                    x_sb = xin.tile([P, KF, B], bf, tag="x")
                    nc.sync.dma_start(x_sb[:], xT[r])
                    ps0 = pp.tile([P, NG, B], f32, tag="ps0")
                    for g in GATE_ORDER:
                        for k in range(KF):
                            nc.tensor.matmul(
                                ps0[:, g, :], w_ih0[:, k, g, :], x_sb[:, k, :],
                                start=(k == 0), stop=False,
                            )
                        for k in range(KH):
                            nc.tensor.matmul(
                                ps0[:, g, :], w_hh0[:, k, g, :], h0T[:, k, :],
                                start=False, stop=(k == KH - 1),
                            )
                    h0new = ew.tile([P, B], bf, tag="h0new")
                    c0 = act_chain("a", ps0, bias0, c0, h0new)
                    nc.sync.dma_start(cc0in[r], h0new[:])
                    nc.gpsimd.collective_compute(
                        "AllGather", mybir.AluOpType.bypass,
                        ins=[cc0in[r]], outs=[cc0out[r]], replica_groups=rg,
                    )
                    h0T_new = hbuf.tile([P, KH, B], bf, tag="h0T")
                    src0 = cc0out[r].rearrange("(c p) b -> p c b", p=P)
                    nc.sync.dma_start(h0T_new[:, 0:4, :], src0[:, 0:4, :])
                    nc.sync.dma_start(h0T_new[:, 4:8, :], src0[:, 4:8, :])
                if r >= 1:
                    # ---- L1 step r-1: uses h0T (h0_{r-1}), h1T (h1_{r-2}) ----
                    ps1 = pp.tile([P, NG, B], f32, tag="ps1")
                    for g in GATE_ORDER:
                        for k in range(KH):
                            nc.tensor.matmul(
                                ps1[:, g, :], w_hh1[:, k, g, :], h1T[:, k, :],
                                start=(k == 0), stop=False,
                            )
                        for k in range(KH):
                            nc.tensor.matmul(
                                ps1[:, g, :], w_ih1[:, k, g, :], h0T[:, k, :],
                                start=False, stop=(k == KH - 1),
                            )
                    h1new = ew.tile([P, B], bf, tag="h1new")
                    c1 = act_chain("b", ps1, bias1, c1, h1new)
                    nc.sync.dma_start(yT[r - 1], h1new[:])
                    if r < nsteps:
                        nc.sync.dma_start(cc1in[r - 1], h1new[:])
                        nc.gpsimd.collective_compute(
                            "AllGather", mybir.AluOpType.bypass,
                            ins=[cc1in[r - 1]], outs=[cc1out[r - 1]],
                            replica_groups=rg,
                        )
                        h1T_new = hbuf.tile([P, KH, B], bf, tag="h1T")
                        src1 = cc1out[r - 1].rearrange("(c p) b -> p c b", p=P)
                        nc.sync.dma_start(h1T_new[:, 0:4, :], src1[:, 0:4, :])
                        nc.sync.dma_start(h1T_new[:, 4:8, :], src1[:, 4:8, :])
                if h0T_new is not None:
                    h0T = h0T_new
                if h1T_new is not None:
                    h1T = h1T_new

    nc.compile()
    return nc


def _prep_w(W, j):
    """W [4H, K] -> [P, KC, NG, P] bf16 with Wt[p,k,g,u] = W[tg*H+j*P+u, k*P+p]."""
    import ml_dtypes

    K = W.shape[1]
    kc = K // P
    out = np.empty((P, kc, NG, P), np.float32)
    for g, tg in enumerate(TORCH_GATE):
        blk = W[tg * H + j * P : tg * H + (j + 1) * P, :]  # [P(u), K]
        out[:, :, g, :] = blk.T.reshape(kc, P, P).transpose(1, 0, 2)
    return np.ascontiguousarray(out.astype(ml_dtypes.bfloat16))


def _prep_b(b_ih, b_hh, j):
    b = (np.asarray(b_ih, np.float32) + np.asarray(b_hh, np.float32))
    out = np.empty((P, NG), np.float32)
    for g, tg in enumerate(TORCH_GATE):
        out[:, g] = b[tg * H + j * P : tg * H + (j + 1) * P]
    return np.ascontiguousarray(out)


def _prep_inputs(x, W_ih0, W_hh0, b_ih0, b_hh0, W_ih1, W_hh1, b_ih1, b_hh1):
    import ml_dtypes

    x = np.asarray(x, np.float32)
    # xT[t, p, k, b] = x[b, t, k*P + p]
    xT = np.ascontiguousarray(
        x.transpose(1, 2, 0).reshape(S, KF, P, B).transpose(0, 2, 1, 3)
        .astype(ml_dtypes.bfloat16)
    )
    in_maps = []
    for j in range(R):
        in_maps.append({
            "xT": xT,
            "wih0": _prep_w(np.asarray(W_ih0, np.float32), j),
            "whh0": _prep_w(np.asarray(W_hh0, np.float32), j),
            "wih1": _prep_w(np.asarray(W_ih1, np.float32), j),
            "whh1": _prep_w(np.asarray(W_hh1, np.float32), j),
            "b0": _prep_b(b_ih0, b_hh0, j),
            "b1": _prep_b(b_ih1, b_hh1, j),
        })
    return in_maps


def _run(nsteps, inputs, trace=False, trace_kwargs=None):
    from concourse.bass_utils import run_bass_kernel_spmd

    if nsteps not in _CACHE:
        _CACHE[nsteps] = _build(nsteps)
    nc = _CACHE[nsteps]
    in_maps = _prep_inputs(**inputs)
    kw = {}
    if trace:
        kw = dict(trace=True, trace_kwargs=trace_kwargs or {})
    res = run_bass_kernel_spmd(nc, in_maps, core_ids=list(range(R)), **kw)
    # yT_j [S, P, B] bf16;  y[b, t, j*P + u] = yT_j[t, u, b]
    ycat = np.stack(
        [np.asarray(res.results[j]["yT"][:nsteps], np.float32) for j in range(R)]
    )  # [R, ns, P, B]
    y = np.ascontiguousarray(np.transpose(ycat, (3, 1, 0, 2)).reshape(B, nsteps, H))
    return y, res


def kernel(**inputs):
    y, _ = _run(S, inputs)
    return y
